# revision 1
# baseline (speedup 1.0000x reference)
"""CrossAttentionBlock Trainium2 kernel.

Shapes (hardcoded): x (16, 512, 64, 64) f32, context (16, 77, 768) f32.
Sharding: data-parallel over batch B=16 across 8 cores (2 batches/core).
Each core runs the full block (groupnorm -> q proj / layernorm -> k,v proj
-> cross attention -> out proj -> +residual) on its 2 batches. No
collectives; weights replicated, outputs gathered on host.

Device layouts per batch:
  x_sb    [128, 4, 4096]  f32   channel c = co*128+p on partitions
  xn      [128, 4, 4096]  bf16  groupnormed x
  qT      [128, 4, 512]   bf16  per pixel-chunk: q.T  (c, pix)
  kT      [128, 4, 77]    bf16  k.T (c, s)
  v_sc    [77, 512]       bf16  v   (s, c)
  attn.T  [77, 512] psum  per (head-pair chunk, pixel-chunk), heads packed
                          2-per-128-partitions via PE row/col tiling.
Softmax is computed unnormalized (logits are small, no max subtraction);
the denominator is materialized partition-replicated via an all-ones
stationary matmul and divided out during AV psum evacuation.
"""

import numpy as np
import ml_dtypes

import concourse.bass as bass
import concourse.tile as tile
from concourse import bacc
from concourse import mybir
from concourse.bass_utils import run_bass_kernel_spmd

F32 = mybir.dt.float32
BF16 = mybir.dt.bfloat16
AF = mybir.ActivationFunctionType
ALU = mybir.AluOpType

B, C, H, W = 16, 512, 64, 64
HW = H * W
S, CTX = 77, 768
HEADS = 8
HD = C // HEADS  # 64
GROUPS = 32
EPS = 1e-5
NCORES = 8
BPC = B // NCORES  # 2 batches per core
P = 128
NCH = HW // 512  # 8 pixel chunks of 512
KQ = C // P      # 4 chunks of 128 for C-contraction
KC = CTX // P    # 6 chunks for CTX-contraction
SCALE = HD ** (-0.5)


def _build_batch(nc, tc, pools, consts, b, xr, ctxr, outr):
    (x_pool, xr_pool, xn_pool, small_pool, q_pool, o_pool, exp_pool, rc_pool,
     fin_pool, ps_mm, ps_qk, ps_d, ps_av) = pools

    # ---- stream x chunks: stats + bias-folded bf16 residual copy ----
    gst = small_pool.tile([P, KQ, 8, 6], F32, tag="gst")
    mv_c = small_pool.tile([P, KQ, 2], F32, tag="mvc")
    x_t = []
    for co in range(KQ):
        xf = x_pool.tile([P, HW], F32, tag="xf")
        nc.sync.dma_start(xf, xr[b, :, co, :])
        for sg in range(8):
            nc.vector.bn_stats(gst[:, co, sg, :], xf[:, sg * 512:(sg + 1) * 512])
        nc.vector.bn_aggr(mv_c[:, co, :], gst[:, co])
        xres = xr_pool.tile([P, HW], BF16, tag=f"xr{co}")
        nc.scalar.activation(xres, xf, AF.Identity,
                             bias=consts["ob"][:, co:co + 1], scale=1.0)
        x_t.append(xres)

    # ---- context layernorm ----
    ctx_t = small_pool.tile([S, CTX], F32, tag="ctx")
    nc.sync.dma_start(ctx_t, ctxr[b])
    lst = small_pool.tile([S, 3, 6], F32, tag="lst")
    for i in range(3):
        nc.vector.bn_stats(lst[:, i, :], ctx_t[:, i * 256:(i + 1) * 256])
    lmv = small_pool.tile([S, 2], F32, tag="lmv")
    nc.vector.bn_aggr(lmv, lst)
    nc.scalar.activation(lmv[:, 1:2], lmv[:, 1:2], AF.Sqrt,
                         bias=consts["eps77"], scale=1.0)
    nc.vector.reciprocal_approx_fast(out=lmv[:, 1:2], in_=lmv[:, 1:2])
    cn_t = small_pool.tile([S, CTX], F32, tag="cn")
    nc.vector.tensor_scalar(cn_t, ctx_t, lmv[:, 0:1], lmv[:, 1:2],
                            ALU.subtract, ALU.mult)
    nc.vector.tensor_mul(cn_t, cn_t, consts["lnw_bc"])
    nc.vector.tensor_add(cn_t, cn_t, consts["lnb_bc"])

    # ---- transpose cn -> cnT [128, 6, 77] bf16 ----
    cnT = small_pool.tile([P, KC, S], BF16, tag="cnT")
    for kc in range(KC):
        pt = ps_d.tile([P, S], F32, tag="pd")
        nc.tensor.transpose(pt, cn_t[:, kc * P:(kc + 1) * P],
                            consts["ident"][:S, :S])
        nc.vector.tensor_copy(cnT[:, kc, :], pt)

    # ---- k projection -> kT [128, 4, 77] bf16 (c on partitions) ----
    kT = small_pool.tile([P, KQ, S], BF16, tag="kT")
    for mo in range(KQ):
        pk = ps_av.tile([P, S], F32, tag="pav")
        for kc in range(KC):
            nc.tensor.matmul(pk, consts["kwT"][:, kc, mo * P:(mo + 1) * P],
                             cnT[:, kc, :], start=(kc == 0), stop=(kc == KC - 1))
        nc.scalar.activation(kT[:, mo, :], pk, AF.Identity,
                             bias=consts["kb"][:, mo:mo + 1], scale=1.0)

    # ---- v projection -> v_sc [77, 512] bf16 (s on partitions) ----
    pv = ps_qk.tile([S, C], F32, tag="pa")
    for kc in range(KC):
        nc.tensor.matmul(pv, cnT[:, kc, :], consts["vwT"][:, kc, :],
                         start=(kc == 0), stop=(kc == KC - 1))
    v_sc = small_pool.tile([S, C], BF16, tag="vsc")
    nc.vector.tensor_add(v_sc, pv, consts["vb_bc"])

    # ---- groupnorm stats combine ----
    t3 = small_pool.tile([P, KQ, 3], F32, tag="t3")
    nc.vector.tensor_copy(t3[:, :, 0:2], mv_c)
    nc.vector.tensor_mul(t3[:, :, 2:3], mv_c[:, :, 0:1], mv_c[:, :, 0:1])
    pg = ps_d.tile([GROUPS // KQ, KQ * 3], F32, tag="pd")
    nc.tensor.matmul(pg, consts["ind1"],
                     t3.rearrange("p a b -> p (a b)"), start=True, stop=True)
    g_sb = small_pool.tile([GROUPS // KQ, KQ, 3], F32, tag="gsb")
    nc.vector.tensor_copy(g_sb.rearrange("p a b -> p (a b)"), pg)
    # stats2: [8, 2, 4] = [group-in-chunk, (mean|rstd), co]
    stats2 = small_pool.tile([GROUPS // KQ, 2, KQ], F32, tag="st2")
    nc.vector.tensor_copy(stats2[:, 0, :], g_sb[:, :, 0])
    vt = small_pool.tile([GROUPS // KQ, KQ], F32, tag="vt")
    nc.vector.tensor_add(vt, g_sb[:, :, 1], g_sb[:, :, 2])
    m2 = small_pool.tile([GROUPS // KQ, KQ], F32, tag="m2")
    nc.vector.tensor_mul(m2, g_sb[:, :, 0], g_sb[:, :, 0])
    nc.vector.tensor_sub(vt, vt, m2)
    nc.scalar.activation(vt, vt, AF.Sqrt, bias=consts["eps8"], scale=1.0)
    nc.vector.reciprocal_approx_fast(out=stats2[:, 1, :], in_=vt)
    pbc = ps_av.tile([P, 2 * KQ], F32, tag="pav")
    nc.tensor.matmul(pbc, consts["ind2"],
                     stats2.rearrange("p a b -> p (a b)"), start=True, stop=True)
    sbc = small_pool.tile([P, 2, KQ], F32, tag="sbc")
    nc.vector.tensor_copy(sbc.rearrange("p a b -> p (a b)"), pbc)
    scale_c = small_pool.tile([P, KQ], F32, tag="scl")
    shift_c = small_pool.tile([P, KQ], F32, tag="shf")
    nc.vector.tensor_mul(scale_c, sbc[:, 1, :], consts["gnw"])
    nc.vector.tensor_mul(shift_c, sbc[:, 0, :], scale_c)
    nc.vector.tensor_sub(shift_c, consts["gnb"], shift_c)
    # apply reads the bias-folded residual copy: xn = xres*scale + shift2,
    # shift2 = shift - ob*scale
    shift2 = small_pool.tile([P, KQ], F32, tag="sh2")
    nc.vector.tensor_mul(shift2, consts["ob"], scale_c)
    nc.vector.tensor_sub(shift2, shift_c, shift2)

    # ---- attention + out proj, per 512-pixel chunk ----
    for n in range(NCH):
        nsl = slice(n * 512, (n + 1) * 512)
        # groupnorm apply (JIT, bf16 4x) + q projection
        xnt = xn_pool.tile([P, KQ, 512], BF16, tag="xnt")
        for kc in range(KQ):
            nc.vector.tensor_scalar(xnt[:, kc, :], x_t[kc][:, nsl],
                                    scale_c[:, kc:kc + 1],
                                    shift2[:, kc:kc + 1],
                                    ALU.mult, ALU.add)
        qT = q_pool.tile([P, KQ, 512], BF16, tag="qT")
        for mo in range(KQ):
            pq = ps_mm.tile([P, 512], F32, tag="pmm")
            for kc in range(KQ):
                nc.tensor.matmul(pq, consts["qwT"][:, kc, mo * P:(mo + 1) * P],
                                 xnt[:, kc, :], start=(kc == 0),
                                 stop=(kc == KQ - 1))
            nc.scalar.activation(qT[:, mo, :], pq, AF.Identity,
                                 bias=consts["qb"][:, mo:mo + 1], scale=1.0)

        outT = o_pool.tile([P, KQ, 512], BF16, tag="outT")
        for co in range(KQ):
            # QK^T for the two heads in this channel chunk (row-group packed)
            pa = ps_qk.tile([S, 2, 512], F32, tag="pa")
            nc.tensor.matmul(pa[:, 0, :], kT[0:HD, co, :], qT[0:HD, co, :],
                             start=True, stop=True, tile_position=(0, 0))
            nc.tensor.matmul(pa[:, 1, :], kT[HD:P, co, :], qT[HD:P, co, :],
                             start=True, stop=True, tile_position=(64, 0))
            ex = exp_pool.tile([S, 2, 512], BF16, tag="ex")
            nc.scalar.activation(ex, pa, AF.Exp, scale=SCALE)
            # denominators, partition-replicated (col-group packed)
            pd = ps_d.tile([P, 512], F32, tag="pd")
            nc.tensor.matmul(pd[0:HD, :], consts["ones77"], ex[:, 0, :],
                             start=True, stop=True, tile_position=(0, 0))
            nc.tensor.matmul(pd[HD:P, :], consts["ones77"], ex[:, 1, :],
                             start=True, stop=True, tile_position=(0, 64))
            rc = rc_pool.tile([P, 512], F32, tag="rc")
            nc.vector.reciprocal_approx_fast(out=rc, in_=pd)
            # AV (col-group packed): psum rows 0-63 = head 2co, 64-127 = 2co+1
            pav = ps_av.tile([P, 512], F32, tag="pav")
            h0, h1 = 2 * co, 2 * co + 1
            nc.tensor.matmul(pav[0:HD, :], v_sc[:, h0 * HD:(h0 + 1) * HD],
                             ex[:, 0, :], start=True, stop=True,
                             tile_position=(0, 0))
            nc.tensor.matmul(pav[HD:P, :], v_sc[:, h1 * HD:(h1 + 1) * HD],
                             ex[:, 1, :], start=True, stop=True,
                             tile_position=(0, 64))
            nc.vector.tensor_mul(outT[:, co, :], pav, rc)

        # out projection + bias + residual -> DRAM
        for mo in range(KQ):
            po = ps_mm.tile([P, 512], F32, tag="pmm")
            for kc in range(KQ):
                nc.tensor.matmul(po, consts["owT"][:, kc, mo * P:(mo + 1) * P],
                                 outT[:, kc, :], start=(kc == 0),
                                 stop=(kc == KQ - 1))
            fin = fin_pool.tile([P, 512], F32, tag="fin")
            nc.vector.tensor_add(fin, po, x_t[mo][:, nsl])
            nc.sync.dma_start(outr[b, :, mo, nsl], fin)


def build_nc(reps=1, loop_reps=0):
    nc = bacc.Bacc()

    x = nc.dram_tensor("x", [BPC, C, HW], F32, kind="ExternalInput")
    ctx_in = nc.dram_tensor("ctx", [BPC, S, CTX], F32, kind="ExternalInput")
    qwT = nc.dram_tensor("qwT", [C, C], BF16, kind="ExternalInput")
    kwT = nc.dram_tensor("kwT", [CTX, C], BF16, kind="ExternalInput")
    vwT = nc.dram_tensor("vwT", [CTX, C], BF16, kind="ExternalInput")
    owT = nc.dram_tensor("owT", [C, C], BF16, kind="ExternalInput")
    qb = nc.dram_tensor("qb", [C], F32, kind="ExternalInput")
    kb = nc.dram_tensor("kb", [C], F32, kind="ExternalInput")
    vb = nc.dram_tensor("vb", [C], F32, kind="ExternalInput")
    ob = nc.dram_tensor("ob", [C], F32, kind="ExternalInput")
    gnw = nc.dram_tensor("gnw", [C], F32, kind="ExternalInput")
    gnb = nc.dram_tensor("gnb", [C], F32, kind="ExternalInput")
    lnw = nc.dram_tensor("lnw", [CTX], F32, kind="ExternalInput")
    lnb = nc.dram_tensor("lnb", [CTX], F32, kind="ExternalInput")
    ident = nc.dram_tensor("ident", [P, P], F32, kind="ExternalInput")
    ones77 = nc.dram_tensor("ones77", [S, HD], BF16, kind="ExternalInput")
    ind1 = nc.dram_tensor("ind1", [P, GROUPS // KQ], F32, kind="ExternalInput")
    ind2 = nc.dram_tensor("ind2", [GROUPS // KQ, P], F32, kind="ExternalInput")
    out = nc.dram_tensor("out", [BPC, C, HW], F32, kind="ExternalOutput")

    xr = x[:].rearrange("b (co p) hw -> b p co hw", p=P)
    ctxr = ctx_in[:]
    outr = out[:].rearrange("b (co p) hw -> b p co hw", p=P)

    with tile.TileContext(nc) as tc:
        with (
            tc.tile_pool(name="singles", bufs=1) as singles,
            tc.tile_pool(name="xp", bufs=2) as x_pool,
            tc.tile_pool(name="xr", bufs=2) as xr_pool,
            tc.tile_pool(name="xnp", bufs=3) as xn_pool,
            tc.tile_pool(name="small", bufs=2) as small_pool,
            tc.tile_pool(name="qp", bufs=2) as q_pool,
            tc.tile_pool(name="op", bufs=2) as o_pool,
            tc.tile_pool(name="expp", bufs=3) as exp_pool,
            tc.tile_pool(name="rcp", bufs=2) as rc_pool,
            tc.tile_pool(name="finp", bufs=2) as fin_pool,
            tc.tile_pool(name="ps_mm", bufs=2, space="PSUM") as ps_mm,
            tc.tile_pool(name="ps_qk", bufs=2, space="PSUM") as ps_qk,
            tc.tile_pool(name="ps_d", bufs=1, space="PSUM") as ps_d,
            tc.tile_pool(name="ps_av", bufs=1, space="PSUM") as ps_av,
        ):
            consts = {}
            t = singles.tile([P, KQ, C], BF16, tag="qwT")
            nc.sync.dma_start(t, qwT[:].rearrange("(ko kp) o -> kp ko o", kp=P))
            consts["qwT"] = t
            t = singles.tile([P, KC, C], BF16, tag="kwT")
            nc.sync.dma_start(t, kwT[:].rearrange("(ko kp) o -> kp ko o", kp=P))
            consts["kwT"] = t
            t = singles.tile([P, KC, C], BF16, tag="vwT")
            nc.sync.dma_start(t, vwT[:].rearrange("(ko kp) o -> kp ko o", kp=P))
            consts["vwT"] = t
            t = singles.tile([P, KQ, C], BF16, tag="owT")
            nc.sync.dma_start(t, owT[:].rearrange("(ko kp) o -> kp ko o", kp=P))
            consts["owT"] = t
            for name, src in (("qb", qb), ("kb", kb), ("ob", ob),
                              ("gnw", gnw), ("gnb", gnb)):
                t = singles.tile([P, KQ], F32, tag=name)
                nc.sync.dma_start(t, src[:].rearrange("(a p) -> p a", p=P))
                consts[name] = t
            t = singles.tile([S, C], F32, tag="vb_bc")
            nc.gpsimd.dma_start(out=t, in_=vb[None, :].to_broadcast([S, C]))
            consts["vb_bc"] = t
            for name, src in (("lnw_bc", lnw), ("lnb_bc", lnb)):
                t = singles.tile([S, CTX], F32, tag=name)
                nc.gpsimd.dma_start(out=t,
                                    in_=src[None, :].to_broadcast([S, CTX]))
                consts[name] = t
            t = singles.tile([P, P], F32, tag="ident")
            nc.sync.dma_start(t, ident[:])
            consts["ident"] = t
            t = singles.tile([S, HD], BF16, tag="ones77")
            nc.sync.dma_start(t, ones77[:])
            consts["ones77"] = t
            t = singles.tile([P, GROUPS // KQ], F32, tag="ind1")
            nc.sync.dma_start(t, ind1[:])
            consts["ind1"] = t
            t = singles.tile([GROUPS // KQ, P], F32, tag="ind2")
            nc.sync.dma_start(t, ind2[:])
            consts["ind2"] = t
            t = singles.tile([S, 1], F32, tag="eps77")
            nc.vector.memset(t, EPS)
            consts["eps77"] = t
            t = singles.tile([GROUPS // KQ, 1], F32, tag="eps8")
            nc.vector.memset(t, EPS)
            consts["eps8"] = t

            pools = (x_pool, xr_pool, xn_pool, small_pool, q_pool, o_pool,
                     exp_pool, rc_pool, fin_pool, ps_mm, ps_qk, ps_d, ps_av)
            if loop_reps:
                with tc.For_i(0, loop_reps, 1):
                    for b in range(BPC):
                        _build_batch(nc, tc, pools, consts, b, xr, ctxr, outr)
            else:
                for _rep in range(reps):
                    for b in range(BPC):
                        _build_batch(nc, tc, pools, consts, b, xr, ctxr, outr)

    nc.finalize()
    return nc


_NC_CACHE = None


def _get_nc():
    global _NC_CACHE
    if _NC_CACHE is None:
        _NC_CACHE = build_nc()
    return _NC_CACHE


def _host_consts():
    bf = ml_dtypes.bfloat16
    g = GROUPS // KQ  # 8 groups per 128-channel chunk
    ind1 = np.zeros((P, g), np.float32)
    for p in range(P):
        ind1[p, p // 16] = 1.0 / 16.0
    ind2 = np.zeros((g, P), np.float32)
    for p in range(P):
        ind2[p // 16, p] = 1.0
    return {
        "ident": np.eye(P, dtype=np.float32),
        "ones77": np.ones((S, HD), dtype=bf),
        "ind1": ind1,
        "ind2": ind2,
    }


def kernel(x, context, gn_w, gn_b, ln_w, ln_b, q_w, q_b, k_w, k_b,
           v_w, v_b, out_w, out_b):
    bf = ml_dtypes.bfloat16
    x = np.asarray(x, np.float32).reshape(B, C, HW)
    context = np.ascontiguousarray(np.asarray(context, np.float32))
    shared = {
        "qwT": np.ascontiguousarray(np.asarray(q_w, np.float32).T).astype(bf),
        "kwT": np.ascontiguousarray(np.asarray(k_w, np.float32).T).astype(bf),
        "vwT": np.ascontiguousarray(np.asarray(v_w, np.float32).T).astype(bf),
        "owT": np.ascontiguousarray(np.asarray(out_w, np.float32).T).astype(bf),
        "qb": np.asarray(q_b, np.float32),
        "kb": np.asarray(k_b, np.float32),
        "vb": np.asarray(v_b, np.float32),
        "ob": np.asarray(out_b, np.float32),
        "gnw": np.asarray(gn_w, np.float32),
        "gnb": np.asarray(gn_b, np.float32),
        "lnw": np.asarray(ln_w, np.float32),
        "lnb": np.asarray(ln_b, np.float32),
        **_host_consts(),
    }
    in_maps = []
    for i in range(NCORES):
        m = dict(shared)
        m["x"] = np.ascontiguousarray(x[i * BPC:(i + 1) * BPC])
        m["ctx"] = np.ascontiguousarray(context[i * BPC:(i + 1) * BPC])
        in_maps.append(m)

    nc = _get_nc()
    res = run_bass_kernel_spmd(nc, in_maps, core_ids=list(range(NCORES)))
    outs = [r["out"] for r in res.results]
    return np.concatenate(outs, axis=0).reshape(B, C, H, W)


if __name__ == "__main__":
    rng = np.random.default_rng(0)
    inputs = {
        "x": rng.standard_normal((B, C, H, W), np.float32),
        "context": rng.standard_normal((B, S, CTX), np.float32),
        "gn_w": np.ones(C, np.float32), "gn_b": np.zeros(C, np.float32),
        "ln_w": np.ones(CTX, np.float32), "ln_b": np.zeros(CTX, np.float32),
        "q_w": rng.standard_normal((C, C), np.float32) * 0.02,
        "q_b": np.zeros(C, np.float32),
        "k_w": rng.standard_normal((C, CTX), np.float32) * 0.02,
        "k_b": np.zeros(C, np.float32),
        "v_w": rng.standard_normal((C, CTX), np.float32) * 0.02,
        "v_b": np.zeros(C, np.float32),
        "out_w": rng.standard_normal((C, C), np.float32) * 0.02,
        "out_b": np.zeros(C, np.float32),
    }
    out = kernel(**inputs)
    print(out.shape, out.dtype)



# revision 6
# speedup vs baseline: 1.2734x; 1.2734x over previous
"""CrossAttentionBlock Trainium2 kernel (v2: skewed pipeline).

Shapes (hardcoded): x (16, 512, 64, 64) f32, context (16, 77, 768) f32.
Sharding: data-parallel over batch B=16 across 8 cores (2 batches/core).
Each core runs the full block on its 2 batches; weights replicated,
outputs gathered on host. No collectives.

v2 structure (vs v1): DMA emission order prioritizes ctx (tiny) then
batch-0 x then batch-1 x so the PE gets context-side matmul work early
and batch-0 stats complete ASAP; groupnorm stats run per 2048-pixel
sub-DMA; the 16 (batch, chunk) units flow through a 3-stage skewed
software pipeline (A: groupnorm-apply + q-proj two steps ahead,
B: attention, C: out-proj one behind) so the in-order PE queue always
has ready work and stays at full p-state.

Softmax is unnormalized (logits small, no max subtraction); the
denominator is formed partition-replicated by an all-ones stationary
matmul and divided out during AV psum evacuation.
"""

import numpy as np
import ml_dtypes

import concourse.bass as bass
import concourse.tile as tile
from concourse import bacc
from concourse import mybir
from concourse.bass_utils import run_bass_kernel_spmd

F32 = mybir.dt.float32
BF16 = mybir.dt.bfloat16
AF = mybir.ActivationFunctionType
ALU = mybir.AluOpType

B, C, H, W = 16, 512, 64, 64
HW = H * W
S, CTX = 77, 768
HEADS = 8
HD = C // HEADS  # 64
GROUPS = 32
EPS = 1e-5
NCORES = 8
BPC = B // NCORES  # 2 batches per core
P = 128
NCH = HW // 512  # 8 pixel chunks of 512
KQ = C // P      # 4 chunks of 128 for C-contraction
KC = CTX // P    # 6 chunks for CTX-contraction
SCALE = HD ** (-0.5)
NU = BPC * NCH   # 16 pipeline units


def _ctx_head(nc, pools, consts, b, ctxr, st):
    """LayerNorm(context) -> cnT -> kT, v_sc for batch b (all PE-light)."""
    small_pool, ps_d, ps_av = pools["small"], pools["ps_d"], pools["ps_av"]
    ctx_t = small_pool.tile([S, CTX], F32, tag=f"ctx{b}")
    nc.sync.dma_start(ctx_t, ctxr[b])
    lst = small_pool.tile([S, 3, 6], F32, tag=f"lst{b}")
    for i in range(3):
        nc.vector.bn_stats(lst[:, i, :], ctx_t[:, i * 256:(i + 1) * 256])
    lmv = small_pool.tile([S, 2], F32, tag=f"lmv{b}")
    nc.vector.bn_aggr(lmv, lst)
    nc.scalar.activation(lmv[:, 1:2], lmv[:, 1:2], AF.Sqrt,
                         bias=consts["eps77"], scale=1.0)
    nc.vector.reciprocal_approx_fast(out=lmv[:, 1:2], in_=lmv[:, 1:2])
    cn_t = small_pool.tile([S, CTX], F32, tag=f"cn{b}")
    nc.vector.tensor_scalar(cn_t, ctx_t, lmv[:, 0:1], lmv[:, 1:2],
                            ALU.subtract, ALU.mult)
    nc.vector.tensor_mul(cn_t, cn_t, consts["lnw_bc"])
    nc.vector.tensor_add(cn_t, cn_t, consts["lnb_bc"])

    # transpose cn -> cnT [128, 6, 77] bf16
    cnT = small_pool.tile([P, KC, S], BF16, tag=f"cnT{b}")
    for kc in range(KC):
        pt = ps_d.tile([P, S], F32, tag="pd")
        nc.tensor.transpose(pt, cn_t[:, kc * P:(kc + 1) * P],
                            consts["ident"][:S, :S])
        nc.vector.tensor_copy(cnT[:, kc, :], pt)

    # k projection -> kT [128, 4, 77] bf16 (c on partitions)
    kT = small_pool.tile([P, KQ, S], BF16, tag=f"kT{b}")
    for mo in range(KQ):
        pk = ps_av.tile([P, S], F32, tag="pav")
        for kc in range(KC):
            nc.tensor.matmul(pk, consts["kwT"][:, kc, mo * P:(mo + 1) * P],
                             cnT[:, kc, :], start=(kc == 0), stop=(kc == KC - 1))
        nc.scalar.activation(kT[:, mo, :], pk, AF.Identity,
                             bias=consts["kb"][:, mo:mo + 1], scale=1.0)

    # v projection -> v_sc [77, 512] bf16 (s on partitions)
    pv = pools["ps_qk"].tile([S, C], F32, tag="pa")
    for kc in range(KC):
        nc.tensor.matmul(pv, cnT[:, kc, :], consts["vwT"][:, kc, :],
                         start=(kc == 0), stop=(kc == KC - 1))
    v_sc = small_pool.tile([S, C], BF16, tag=f"vsc{b}")
    nc.vector.tensor_add(v_sc, pv, consts["vb_bc"])

    st[b]["kT"] = kT
    st[b]["v_sc"] = v_sc


def _x_load_stats(nc, pools, consts, b, xr, st):
    """Stream x for batch b in 2048-pixel halves; bn_stats per 512 slice;
    bias-folded bf16 residual copies."""
    small_pool, x_pool, xr_pool = pools["small"], pools["x"], pools["xr"]
    gst = small_pool.tile([P, KQ, 8, 6], F32, tag=f"gst{b}")
    mv_c = small_pool.tile([P, KQ, 2], F32, tag=f"mvc{b}")
    x_t = []
    for co in range(KQ):
        xres = xr_pool.tile([P, HW], BF16, tag=f"xr{b}_{co}")
        x_t.append(xres)
    for co in range(KQ):
        for h in range(2):
            hsl = slice(h * 2048, (h + 1) * 2048)
            xf = x_pool.tile([P, 2048], F32, tag="xf")
            nc.sync.dma_start(xf, xr[b, :, co, hsl])
            for sg in range(4):
                g = h * 4 + sg
                nc.vector.bn_stats(gst[:, co, g, :],
                                   xf[:, sg * 512:(sg + 1) * 512])
            nc.scalar.activation(x_t[co][:, hsl], xf, AF.Identity,
                                 bias=consts["ob"][:, co:co + 1], scale=1.0)
        nc.vector.bn_aggr(mv_c[:, co, :], gst[:, co])
    st[b]["x_t"] = x_t
    st[b]["mv_c"] = mv_c


def _stats_combine(nc, pools, consts, b, st):
    """Combine per-partition stats into per-group scale/shift (as v1)."""
    small_pool, ps_d, ps_av = pools["small"], pools["ps_d"], pools["ps_av"]
    mv_c = st[b]["mv_c"]
    t3 = small_pool.tile([P, KQ, 3], F32, tag=f"t3{b}")
    nc.vector.tensor_copy(t3[:, :, 0:2], mv_c)
    nc.vector.tensor_mul(t3[:, :, 2:3], mv_c[:, :, 0:1], mv_c[:, :, 0:1])
    pg = ps_d.tile([GROUPS // KQ, KQ * 3], F32, tag="pd")
    nc.tensor.matmul(pg, consts["ind1"],
                     t3.rearrange("p a b -> p (a b)"), start=True, stop=True)
    g_sb = small_pool.tile([GROUPS // KQ, KQ, 3], F32, tag=f"gsb{b}")
    nc.vector.tensor_copy(g_sb.rearrange("p a b -> p (a b)"), pg)
    stats2 = small_pool.tile([GROUPS // KQ, 2, KQ], F32, tag=f"st2{b}")
    nc.vector.tensor_copy(stats2[:, 0, :], g_sb[:, :, 0])
    vt = small_pool.tile([GROUPS // KQ, KQ], F32, tag=f"vt{b}")
    nc.vector.tensor_add(vt, g_sb[:, :, 1], g_sb[:, :, 2])
    m2 = small_pool.tile([GROUPS // KQ, KQ], F32, tag=f"m2{b}")
    nc.vector.tensor_mul(m2, g_sb[:, :, 0], g_sb[:, :, 0])
    nc.vector.tensor_sub(vt, vt, m2)
    nc.scalar.activation(vt, vt, AF.Sqrt, bias=consts["eps8"], scale=1.0)
    nc.vector.reciprocal_approx_fast(out=stats2[:, 1, :], in_=vt)
    pbc = ps_av.tile([P, 2 * KQ], F32, tag="pav")
    nc.tensor.matmul(pbc, consts["ind2"],
                     stats2.rearrange("p a b -> p (a b)"), start=True, stop=True)
    sbc = small_pool.tile([P, 2, KQ], F32, tag=f"sbc{b}")
    nc.vector.tensor_copy(sbc.rearrange("p a b -> p (a b)"), pbc)
    scale_c = small_pool.tile([P, KQ], F32, tag=f"scl{b}")
    shift_c = small_pool.tile([P, KQ], F32, tag=f"shf{b}")
    nc.vector.tensor_mul(scale_c, sbc[:, 1, :], consts["gnw"])
    nc.vector.tensor_mul(shift_c, sbc[:, 0, :], scale_c)
    nc.vector.tensor_sub(shift_c, consts["gnb"], shift_c)
    shift2 = small_pool.tile([P, KQ], F32, tag=f"sh2{b}")
    nc.vector.tensor_mul(shift2, consts["ob"], scale_c)
    nc.vector.tensor_sub(shift2, shift_c, shift2)
    st[b]["scale_c"] = scale_c
    st[b]["shift2"] = shift2


def _stage_a_xnt(nc, pools, consts, st, u):
    """Groupnorm apply (vector) for unit u -> xnt."""
    b, n = divmod(u, NCH)
    nsl = slice(n * 512, (n + 1) * 512)
    x_t = st[b]["x_t"]
    scale_c, shift2 = st[b]["scale_c"], st[b]["shift2"]
    xnt = pools["xn"].tile([P, KQ, 512], BF16, tag="xnt")
    for kc in range(KQ):
        nc.vector.tensor_scalar(xnt[:, kc, :], x_t[kc][:, nsl],
                                scale_c[:, kc:kc + 1], shift2[:, kc:kc + 1],
                                ALU.mult, ALU.add)
    return xnt


def _stage_a_q(nc, pools, consts, st, u, xnt):
    """q projection (PE) for unit u -> qT(u)."""
    qT = pools["q"].tile([P, KQ, 512], BF16, tag="qT")
    for mo in range(KQ):
        pq = pools["ps_proj"].tile([P, 512], F32, tag="pmm")
        for kc in range(KQ):
            nc.tensor.matmul(pq, consts["qwT"][:, kc, mo * P:(mo + 1) * P],
                             xnt[:, kc, :], start=(kc == 0), stop=(kc == KQ - 1))
        nc.scalar.activation(qT[:, mo, :], pq, AF.Identity,
                             bias=consts["qb"][:, mo:mo + 1], scale=1.0)
    st["qT"][u] = qT


def _stage_a(nc, pools, consts, st, u):
    xnt = _stage_a_xnt(nc, pools, consts, st, u)
    _stage_a_q(nc, pools, consts, st, u, xnt)


def _stage_b(nc, pools, consts, st, u):
    """Attention for unit u: QK -> exp -> denom -> recip -> AV -> outT(u)."""
    b, n = divmod(u, NCH)
    kT, v_sc = st[b]["kT"], st[b]["v_sc"]
    qT = st["qT"].pop(u)
    ex_l = []
    for co in range(KQ):
        pa = pools["ps_qk"].tile([S, 2, 512], F32, tag="pa")
        nc.tensor.matmul(pa[:, 0, :], kT[0:HD, co, :], qT[0:HD, co, :],
                         start=True, stop=True, tile_position=(0, 0))
        nc.tensor.matmul(pa[:, 1, :], kT[HD:P, co, :], qT[HD:P, co, :],
                         start=True, stop=True, tile_position=(64, 0))
        ex = pools["exp"].tile([S, 2, 512], BF16, tag="ex")
        nc.scalar.activation(ex, pa, AF.Exp, scale=SCALE)
        ex_l.append(ex)
    outT = pools["o"].tile([P, KQ, 512], BF16, tag="outT")
    for co in range(KQ):
        ex = ex_l[co]
        pd = pools["ps_d"].tile([P, 512], F32, tag="pd")
        nc.tensor.matmul(pd[0:HD, :], consts["ones77"], ex[:, 0, :],
                         start=True, stop=True, tile_position=(0, 0))
        nc.tensor.matmul(pd[HD:P, :], consts["ones77"], ex[:, 1, :],
                         start=True, stop=True, tile_position=(0, 64))
        rc = pools["rc"].tile([P, 512], F32, tag="rc")
        nc.vector.reciprocal_approx_fast(out=rc, in_=pd)
        pav = pools["ps_av"].tile([P, 512], F32, tag="pav")
        h0, h1 = 2 * co, 2 * co + 1
        nc.tensor.matmul(pav[0:HD, :], v_sc[:, h0 * HD:(h0 + 1) * HD],
                         ex[:, 0, :], start=True, stop=True,
                         tile_position=(0, 0))
        nc.tensor.matmul(pav[HD:P, :], v_sc[:, h1 * HD:(h1 + 1) * HD],
                         ex[:, 1, :], start=True, stop=True,
                         tile_position=(0, 64))
        nc.vector.tensor_mul(outT[:, co, :], pav, rc)
    st["outT"][u] = outT


def _stage_c(nc, pools, consts, st, u, outr):
    """Out projection + residual + store for unit u."""
    b, n = divmod(u, NCH)
    nsl = slice(n * 512, (n + 1) * 512)
    x_t = st[b]["x_t"]
    outT = st["outT"].pop(u)
    for mo in range(KQ):
        po = pools["ps_proj"].tile([P, 512], F32, tag="pmm")
        for kc in range(KQ):
            nc.tensor.matmul(po, consts["owT"][:, kc, mo * P:(mo + 1) * P],
                             outT[:, kc, :], start=(kc == 0),
                             stop=(kc == KQ - 1))
        fin = pools["fin"].tile([P, 512], F32, tag="fin")
        nc.vector.tensor_add(fin, po, x_t[mo][:, nsl])
        nc.sync.dma_start(outr[b, :, mo, nsl], fin)


def build_nc(reps=1, loop_reps=0):
    nc = bacc.Bacc()

    x = nc.dram_tensor("x", [BPC, C, HW], F32, kind="ExternalInput")
    ctx_in = nc.dram_tensor("ctx", [BPC, S, CTX], F32, kind="ExternalInput")
    qwT = nc.dram_tensor("qwT", [C, C], BF16, kind="ExternalInput")
    kwT = nc.dram_tensor("kwT", [CTX, C], BF16, kind="ExternalInput")
    vwT = nc.dram_tensor("vwT", [CTX, C], BF16, kind="ExternalInput")
    owT = nc.dram_tensor("owT", [C, C], BF16, kind="ExternalInput")
    qb = nc.dram_tensor("qb", [C], F32, kind="ExternalInput")
    kb = nc.dram_tensor("kb", [C], F32, kind="ExternalInput")
    vb = nc.dram_tensor("vb", [C], F32, kind="ExternalInput")
    ob = nc.dram_tensor("ob", [C], F32, kind="ExternalInput")
    gnw = nc.dram_tensor("gnw", [C], F32, kind="ExternalInput")
    gnb = nc.dram_tensor("gnb", [C], F32, kind="ExternalInput")
    lnw = nc.dram_tensor("lnw", [CTX], F32, kind="ExternalInput")
    lnb = nc.dram_tensor("lnb", [CTX], F32, kind="ExternalInput")
    ident = nc.dram_tensor("ident", [P, P], F32, kind="ExternalInput")
    ones77 = nc.dram_tensor("ones77", [S, HD], BF16, kind="ExternalInput")
    ind1 = nc.dram_tensor("ind1", [P, GROUPS // KQ], F32, kind="ExternalInput")
    ind2 = nc.dram_tensor("ind2", [GROUPS // KQ, P], F32, kind="ExternalInput")
    out = nc.dram_tensor("out", [BPC, C, HW], F32, kind="ExternalOutput")

    xr = x[:].rearrange("b (co p) hw -> b p co hw", p=P)
    ctxr = ctx_in[:]
    outr = out[:].rearrange("b (co p) hw -> b p co hw", p=P)

    with tile.TileContext(nc) as tc:
        with (
            tc.tile_pool(name="singles", bufs=1) as singles,
            tc.tile_pool(name="xp", bufs=3) as x_pool,
            tc.tile_pool(name="xr", bufs=1) as xr_pool,
            tc.tile_pool(name="xnp", bufs=3) as xn_pool,
            tc.tile_pool(name="small", bufs=1) as small_pool,
            tc.tile_pool(name="qp", bufs=3) as q_pool,
            tc.tile_pool(name="op", bufs=2) as o_pool,
            tc.tile_pool(name="expp", bufs=5) as exp_pool,
            tc.tile_pool(name="rcp", bufs=2) as rc_pool,
            tc.tile_pool(name="finp", bufs=4) as fin_pool,
            tc.tile_pool(name="ps_proj", bufs=2, space="PSUM") as ps_proj,
            tc.tile_pool(name="ps_qk", bufs=2, space="PSUM") as ps_qk,
            tc.tile_pool(name="ps_d", bufs=1, space="PSUM") as ps_d,
            tc.tile_pool(name="ps_av", bufs=1, space="PSUM") as ps_av,
        ):
            pools = {
                "x": x_pool, "xr": xr_pool, "xn": xn_pool, "small": small_pool,
                "q": q_pool, "o": o_pool, "exp": exp_pool, "rc": rc_pool,
                "fin": fin_pool, "ps_proj": ps_proj, "ps_qk": ps_qk,
                "ps_d": ps_d, "ps_av": ps_av,
            }
            consts = {}
            t = singles.tile([P, KQ, C], BF16, tag="qwT")
            nc.sync.dma_start(t, qwT[:].rearrange("(ko kp) o -> kp ko o", kp=P))
            consts["qwT"] = t
            t = singles.tile([P, KC, C], BF16, tag="kwT")
            nc.sync.dma_start(t, kwT[:].rearrange("(ko kp) o -> kp ko o", kp=P))
            consts["kwT"] = t
            t = singles.tile([P, KC, C], BF16, tag="vwT")
            nc.sync.dma_start(t, vwT[:].rearrange("(ko kp) o -> kp ko o", kp=P))
            consts["vwT"] = t
            t = singles.tile([P, KQ, C], BF16, tag="owT")
            nc.sync.dma_start(t, owT[:].rearrange("(ko kp) o -> kp ko o", kp=P))
            consts["owT"] = t
            for name, src in (("qb", qb), ("kb", kb), ("ob", ob),
                              ("gnw", gnw), ("gnb", gnb)):
                t = singles.tile([P, KQ], F32, tag=name)
                nc.sync.dma_start(t, src[:].rearrange("(a p) -> p a", p=P))
                consts[name] = t
            t = singles.tile([S, C], F32, tag="vb_bc")
            nc.gpsimd.dma_start(out=t, in_=vb[None, :].to_broadcast([S, C]))
            consts["vb_bc"] = t
            for name, src in (("lnw_bc", lnw), ("lnb_bc", lnb)):
                t = singles.tile([S, CTX], F32, tag=name)
                nc.gpsimd.dma_start(out=t,
                                    in_=src[None, :].to_broadcast([S, CTX]))
                consts[name] = t
            t = singles.tile([P, P], F32, tag="ident")
            nc.sync.dma_start(t, ident[:])
            consts["ident"] = t
            t = singles.tile([S, HD], BF16, tag="ones77")
            nc.sync.dma_start(t, ones77[:])
            consts["ones77"] = t
            t = singles.tile([P, GROUPS // KQ], F32, tag="ind1")
            nc.sync.dma_start(t, ind1[:])
            consts["ind1"] = t
            t = singles.tile([GROUPS // KQ, P], F32, tag="ind2")
            nc.sync.dma_start(t, ind2[:])
            consts["ind2"] = t
            t = singles.tile([S, 1], F32, tag="eps77")
            nc.vector.memset(t, EPS)
            consts["eps77"] = t
            t = singles.tile([GROUPS // KQ, 1], F32, tag="eps8")
            nc.vector.memset(t, EPS)
            consts["eps8"] = t

            def build_once():
                st = {0: {}, 1: {}, "qT": {}, "outT": {}}
                # context-side work first: tiny DMAs, early PE work
                for b in range(BPC):
                    _ctx_head(nc, pools, consts, b, ctxr, st)
                # batch-0 x stream + stats, then batch-1
                for b in range(BPC):
                    _x_load_stats(nc, pools, consts, b, xr, st)
                    _stats_combine(nc, pools, consts, b, st)
                # skewed pipeline: A two ahead, C at current
                _stage_a(nc, pools, consts, st, 0)
                _stage_a(nc, pools, consts, st, 1)
                for u in range(NU):
                    xnt_next = (_stage_a_xnt(nc, pools, consts, st, u + 2)
                                if u + 2 < NU else None)
                    _stage_b(nc, pools, consts, st, u)
                    if xnt_next is not None:
                        _stage_a_q(nc, pools, consts, st, u + 2, xnt_next)
                    _stage_c(nc, pools, consts, st, u, outr)

            if loop_reps:
                with tc.For_i(0, loop_reps, 1):
                    build_once()
            else:
                for _rep in range(reps):
                    build_once()

    nc.finalize()
    return nc


_NC_CACHE = None


def _get_nc():
    global _NC_CACHE
    if _NC_CACHE is None:
        _NC_CACHE = build_nc()
    return _NC_CACHE


def _host_consts():
    bf = ml_dtypes.bfloat16
    g = GROUPS // KQ  # 8 groups per 128-channel chunk
    ind1 = np.zeros((P, g), np.float32)
    for p in range(P):
        ind1[p, p // 16] = 1.0 / 16.0
    ind2 = np.zeros((g, P), np.float32)
    for p in range(P):
        ind2[p // 16, p] = 1.0
    return {
        "ident": np.eye(P, dtype=np.float32),
        "ones77": np.ones((S, HD), dtype=bf),
        "ind1": ind1,
        "ind2": ind2,
    }


def kernel(x, context, gn_w, gn_b, ln_w, ln_b, q_w, q_b, k_w, k_b,
           v_w, v_b, out_w, out_b):
    bf = ml_dtypes.bfloat16
    x = np.asarray(x, np.float32).reshape(B, C, HW)
    context = np.ascontiguousarray(np.asarray(context, np.float32))
    shared = {
        "qwT": np.ascontiguousarray(np.asarray(q_w, np.float32).T).astype(bf),
        "kwT": np.ascontiguousarray(np.asarray(k_w, np.float32).T).astype(bf),
        "vwT": np.ascontiguousarray(np.asarray(v_w, np.float32).T).astype(bf),
        "owT": np.ascontiguousarray(np.asarray(out_w, np.float32).T).astype(bf),
        "qb": np.asarray(q_b, np.float32),
        "kb": np.asarray(k_b, np.float32),
        "vb": np.asarray(v_b, np.float32),
        "ob": np.asarray(out_b, np.float32),
        "gnw": np.asarray(gn_w, np.float32),
        "gnb": np.asarray(gn_b, np.float32),
        "lnw": np.asarray(ln_w, np.float32),
        "lnb": np.asarray(ln_b, np.float32),
        **_host_consts(),
    }
    in_maps = []
    for i in range(NCORES):
        m = dict(shared)
        m["x"] = np.ascontiguousarray(x[i * BPC:(i + 1) * BPC])
        m["ctx"] = np.ascontiguousarray(context[i * BPC:(i + 1) * BPC])
        in_maps.append(m)

    nc = _get_nc()
    res = run_bass_kernel_spmd(nc, in_maps, core_ids=list(range(NCORES)))
    outs = [r["out"] for r in res.results]
    return np.concatenate(outs, axis=0).reshape(B, C, H, W)


if __name__ == "__main__":
    rng = np.random.default_rng(0)
    inputs = {
        "x": rng.standard_normal((B, C, H, W), np.float32),
        "context": rng.standard_normal((B, S, CTX), np.float32),
        "gn_w": np.ones(C, np.float32), "gn_b": np.zeros(C, np.float32),
        "ln_w": np.ones(CTX, np.float32), "ln_b": np.zeros(CTX, np.float32),
        "q_w": rng.standard_normal((C, C), np.float32) * 0.02,
        "q_b": np.zeros(C, np.float32),
        "k_w": rng.standard_normal((C, CTX), np.float32) * 0.02,
        "k_b": np.zeros(C, np.float32),
        "v_w": rng.standard_normal((C, CTX), np.float32) * 0.02,
        "v_b": np.zeros(C, np.float32),
        "out_w": rng.standard_normal((C, C), np.float32) * 0.02,
        "out_b": np.zeros(C, np.float32),
    }
    out = kernel(**inputs)
    print(out.shape, out.dtype)
